# revision 2
# baseline (speedup 1.0000x reference)
"""BCQConv1D TRN2 kernel v2: out[b,s,o] = x[b,s,:] @ W[o,:]^T + bias[o],
W[o, g*A+a] = sum_qb alpha[o,g,qb] * binary[o,g,a,qb].

Sharding: 2D tensor-parallel - out_features split 4 ways x batch split
2 ways across the 8 NeuronCores (cuts replicated-x HBM traffic 2x vs
pure column-parallel).

Per core: reconstruct the W^T shard ON THE PE via diagonal matmuls
(psum[a,o] += binary_qb[o',a]^T @ diag(alpha[o,g,qb]) accumulated over
the 3 bit planes - fuses scale+transpose+sum, keeping the DVE free),
keep W^T resident in SBUF as bf16, stream x in fat-line (8KB) DMA
slabs, cast to bf16, and run N=512 bf16 matmuls accumulating over
K=4096 in PSUM. Bias is pre-loaded into PSUM with a K=1 ones-matmul
so the PSUM->SBUF eviction is a plain copy on the Act engine.

Host side only shards/relayouts inputs (x transposed/tiled so every
DMA line is 8KB contiguous; binary reshaped + represented as bf16,
which is exact for +-1).
"""

import numpy as np
import ml_dtypes

import concourse.bass as bass
import concourse.tile as tile
from concourse import bacc, mybir
from concourse.bass_utils import run_bass_kernel_spmd
from concourse.masks import make_identity

# Problem shape (hardcoded per contest contract)
B, S, I, O = 4, 2048, 4096, 4096
G, A, QB = 32, 128, 3
BS = B * S  # 8192
P = 128
KT = I // P  # 32 k-tiles (i = g*A + a, A == P)

# Sharding: O_WAYS x BS_WAYS == 8 cores
N_CORES = 8
O_WAYS = 4
BS_WAYS = 2
O_SH = O // O_WAYS  # 1024 out features per core
BS_SH = BS // BS_WAYS  # 4096 batch rows per core
OT = O_SH // P  # 8 o-tiles for recon

NFREE = 512  # matmul moving free dim (one PSUM bank of fp32)
NB = O_SH // NFREE  # 2 o-blocks -> stationary x-tile reused 2x
SC = 256  # bs columns per super (2 psum row-tiles)
NSUP = BS_SH // SC  # 16 supers
NSUB = SC // P  # 2 row-tiles per super
KQ = 8  # k-tiles per x DMA slab (8KB contiguous lines)
NKQ = KT // KQ  # 4 slabs per super
GQ = 8  # groups per binary DMA slab

F32 = mybir.dt.float32
BF16 = mybir.dt.bfloat16
AluOp = mybir.AluOpType


def build_nc():
    nc = bacc.Bacc(target_bir_lowering=False)
    xt_d = nc.declare_dram_parameter("xt", [P, NSUP, KT, SC], F32, isOutput=False)
    alpha_d = nc.declare_dram_parameter("alpha", [O_SH, G, QB], F32, isOutput=False)
    # binary relayout: [ot, gq, o(128), gg, qb, a] in bf16 (exact +-1)
    binary_d = nc.declare_dram_parameter(
        "binary", [OT, G // GQ, P, GQ, QB, A], BF16, isOutput=False
    )
    bias_d = nc.declare_dram_parameter("bias", [O_SH], F32, isOutput=False)
    out_d = nc.declare_dram_parameter("out", [BS_SH, O_SH], F32, isOutput=True)

    with tile.TileContext(nc) as tc:
        with (
            tc.tile_pool(name="const", bufs=1) as cpool,
            tc.tile_pool(name="wt", bufs=1) as wtpool,
            tc.tile_pool(name="bt", bufs=4) as btpool,
            tc.tile_pool(name="dpool", bufs=12) as dpool,
            tc.tile_pool(name="xf", bufs=2) as xf,
            tc.tile_pool(name="xb", bufs=6) as xb,
            tc.tile_pool(name="op", bufs=4) as op,
            tc.tile_pool(name="psum", bufs=8, space="PSUM") as pp,
        ):
            # --- constants ---
            ident = cpool.tile([P, P], BF16, name="ident")
            make_identity(nc, ident)
            ones = cpool.tile([1, P], BF16, name="ones")
            nc.vector.memset(ones, 1.0)
            bias_f32 = cpool.tile([1, O_SH], F32, name="bias_f32")
            nc.sync.dma_start(out=bias_f32, in_=bias_d.ap().unsqueeze(0))
            bias_row = cpool.tile([1, O_SH], BF16, name="bias_row")
            nc.vector.tensor_copy(out=bias_row, in_=bias_f32)
            bias_bc = cpool.tile([P, O_SH], F32, name="bias_bc")
            for j in range(NB):
                pbt = pp.tile([P, NFREE], F32, tag="ps", name=f"psb{j}")
                nc.tensor.matmul(
                    pbt, ones, bias_row[:, j * NFREE : (j + 1) * NFREE],
                    start=True, stop=True,
                )
                nc.vector.tensor_copy(
                    out=bias_bc[:, j * NFREE : (j + 1) * NFREE], in_=pbt
                )

            # --- alpha (per-partition scalars), all o-tiles resident ---
            alpha_sb = []
            for ot in range(OT):
                at = cpool.tile([P, G, QB], F32, name=f"alpha{ot}")
                nc.sync.dma_start(out=at, in_=alpha_d.ap()[ot * P : (ot + 1) * P])
                alpha_sb.append(at)

            # --- W^T shard, resident bf16, single [P, KT, O_SH] tensor ---
            wt_all = wtpool.tile([P, KT, O_SH], BF16, name="wt_all")

            # --- recon on the PE: psum[a,o] += B_qb[o',a]^T @ diag(alpha) ---
            GH = G // 2  # D-tiles split in g-halves for finer pipelining
            for ot in range(OT):
                at = alpha_sb[ot]
                # D_qb[o', (g, o)] = ident[o', o] * alpha[o', g, qb]
                # one DVE op per (ot, qb, g-half)
                d_tiles = [[None, None] for _ in range(QB)]
                for gh in range(2):
                    for qb in range(QB):
                        dt_ = dpool.tile(
                            [P, GH, P], BF16, tag="dq", name=f"d{ot}_{qb}_{gh}"
                        )
                        nc.vector.tensor_tensor(
                            out=dt_,
                            in0=ident[:, :].unsqueeze(1).to_broadcast((P, GH, P)),
                            in1=at[
                                :, gh * GH : (gh + 1) * GH, qb : qb + 1
                            ].to_broadcast((P, GH, P)),
                            op=AluOp.mult,
                        )
                        d_tiles[qb][gh] = dt_
                for gq in range(G // GQ):
                    bt = btpool.tile([P, GQ, QB, A], BF16, tag="bt")
                    dma_eng = nc.sync if gq % 2 == 0 else nc.scalar
                    dma_eng.dma_start(out=bt, in_=binary_d.ap()[ot, gq])
                    # 4-wide psum batches -> one Act copy per 4 groups
                    for gb in range(GQ // 4):
                        ptt4 = pp.tile([P, 4, P], F32, tag="ps", name=f"pr{ot}_{gq}_{gb}")
                        for g4 in range(4):
                            gg = gb * 4 + g4
                            g = gq * GQ + gg
                            gh, gi = divmod(g, GH)
                            for qb in range(QB):
                                nc.tensor.matmul(
                                    ptt4[:, g4],
                                    bt[:, gg, qb, :],
                                    d_tiles[qb][gh][:, gi],
                                    start=(qb == 0),
                                    stop=(qb == QB - 1),
                                )
                        g0 = gq * GQ + gb * 4
                        nc.scalar.copy(
                            out=wt_all[:, g0 : g0 + 4, ot * P : (ot + 1) * P],
                            in_=ptt4,
                        )

            # --- main matmul: out[bs, o] = x^T.T @ W^T (+bias in psum) ---
            for sup in range(NSUP):
                psums = [
                    [
                        pp.tile([P, NFREE], F32, tag="ps", name=f"mm{sup}_{s}_{j}")
                        for j in range(NB)
                    ]
                    for s in range(NSUB)
                ]
                for kq in range(NKQ):
                    xt_f = xf.tile([P, KQ, SC], F32, tag="xf")
                    dma_eng = nc.sync if kq % 2 == 0 else nc.scalar
                    dma_eng.dma_start(
                        out=xt_f,
                        in_=xt_d.ap()[:, sup, kq * KQ : (kq + 1) * KQ, :],
                    )
                    xt_b = xb.tile([P, KQ, SC], BF16, tag="xb")
                    if kq % 2 == 0:
                        nc.vector.tensor_copy(out=xt_b, in_=xt_f)
                    else:
                        nc.scalar.copy(out=xt_b, in_=xt_f)
                    for kk in range(KQ):
                        k = kq * KQ + kk
                        for s in range(NSUB):
                            for j in range(NB):
                                nc.tensor.matmul(
                                    psums[s][j],
                                    xt_b[:, kk, s * P : (s + 1) * P],
                                    wt_all[:, k, j * NFREE : (j + 1) * NFREE],
                                    start=(k == 0),
                                    stop=(k == KT - 1),
                                )
                for s in range(NSUB):
                    os_t = op.tile([P, O_SH], F32, tag="os")
                    for j in range(NB):
                        nc.vector.tensor_tensor(
                            out=os_t[:, j * NFREE : (j + 1) * NFREE],
                            in0=psums[s][j],
                            in1=bias_bc[:, j * NFREE : (j + 1) * NFREE],
                            op=AluOp.add,
                        )
                    dma_eng = nc.sync if s % 2 == 0 else nc.scalar
                    dma_eng.dma_start(
                        out=out_d.ap()[
                            sup * SC + s * P : sup * SC + (s + 1) * P, :
                        ],
                        in_=os_t,
                    )

    if not nc.is_finalized():
        nc.finalize()
    return nc


def shard_inputs(x, alpha, bias, binary):
    """Host-side sharding/relayout only (dtype repack of +-1 binary to
    bf16 is exact)."""
    x2 = np.ascontiguousarray(x).reshape(BS, I)
    alpha = np.ascontiguousarray(alpha)
    bias = np.ascontiguousarray(bias)

    # x relayout per bc shard: [P, NSUP, KT, SC],
    # xtp[p, sup, kt, c] = x2[bc*BS_SH + sup*SC + c, kt*P + p]
    x_shards = []
    for bc in range(BS_WAYS):
        xs = x2[bc * BS_SH : (bc + 1) * BS_SH]  # [BS_SH, I]
        xs = xs.reshape(NSUP, SC, KT, P).transpose(3, 0, 2, 1)
        x_shards.append(np.ascontiguousarray(xs))

    # binary relayout per oc shard: [OT, G//GQ, P, GQ, QB, A] bf16
    b_shards = []
    for oc in range(O_WAYS):
        bsh = binary[oc * O_SH : (oc + 1) * O_SH]  # [O_SH, G, A, QB]
        bsh = bsh.reshape(OT, P, G // GQ, GQ, A, QB).transpose(0, 2, 1, 3, 5, 4)
        b_shards.append(np.ascontiguousarray(bsh.astype(ml_dtypes.bfloat16)))

    in_maps = []
    for c in range(N_CORES):
        oc, bc = divmod(c, BS_WAYS)
        osl = slice(oc * O_SH, (oc + 1) * O_SH)
        in_maps.append(
            {
                "xt": x_shards[bc],
                "alpha": np.ascontiguousarray(alpha[osl]),
                "binary": b_shards[oc],
                "bias": np.ascontiguousarray(bias[osl]),
            }
        )
    return in_maps


def assemble_output(results):
    out = np.empty((BS, O), dtype=np.float32)
    for c in range(N_CORES):
        oc, bc = divmod(c, BS_WAYS)
        out[
            bc * BS_SH : (bc + 1) * BS_SH, oc * O_SH : (oc + 1) * O_SH
        ] = results[c]["out"]
    return out.reshape(B, S, O)


_NC_CACHE = None


def kernel(x, alpha, bias, binary):
    global _NC_CACHE
    if _NC_CACHE is None:
        _NC_CACHE = build_nc()
    nc = _NC_CACHE
    in_maps = shard_inputs(
        np.asarray(x, dtype=np.float32),
        np.asarray(alpha, dtype=np.float32),
        np.asarray(bias, dtype=np.float32),
        np.asarray(binary, dtype=np.float32),
    )
    res = run_bass_kernel_spmd(nc, in_maps, list(range(N_CORES)))
    return assemble_output(res.results)
